# revision 15
# baseline (speedup 1.0000x reference)
"""BatchAllTripletLoss kernel for 8 Trainium2 NeuronCores.

Reference computation:
    pd = pairwise_euclidean(rep)                        # [512, 512]
    tl[a,p,k] = relu(pd[a,p] - pd[a,k] + 5.0) * mask    # [512, 512, 512]
    loss = sum(tl) / (count(tl > eps) + eps)

The mask (p!=a, k!=a, p!=k, label[p]==label[a], label[k]!=label[a])
collapses: label[p]==label[a] and label[k]!=label[a] imply p!=k and k!=a,
so valid triplets are exactly (anchor-positive pairs) x (k with a
different label).  With 64 labels over 512 rows there are only ~4100
(a,p) pairs, so instead of a dense [N,N,N] sweep each core processes its
anchors' pairs as rows of [128-pair, 512-k] tiles:

  per core (64 anchors):
    d[64,512]   = sqrt(relu(aug-matmul))            PE + DVE + ACT
    ym          = d + BIGM*same_label               DVE
    per pair-tile t:
      Gym       = sel_t.T @ ym                      PE one-hot row gather
      x[p]      = sum_k (iota==pidx)*Gym            DVE; = d[a,p] + BIGM
      xp        = x + (margin - BIGM)               DVE
      S_t[p]    = sum_k relu(xp - Gym)              ACT accum
      C_t[p]    = sum_k (Gym < xp)                  DVE accum
    out[1,2*Tp] = ones.T @ [S | C]                  PE partition sum

All matmuls run in float32r (single-pass fp32, ~2^-13 relative rounding;
the one-hot gather then carries that rounding into d).  BIGM = 128 both
masks out same-label k columns (xp <= ~35 << 128 so relu/count give
exactly 0) and carries the bias through the gather; the combined
rounding is ~1e-2 absolute per term, mean-zero, ~1e-4 on the final sums.
rep arrives both row-major (for the row-norm accumulates) and
host-transposed (pure layout permutation) so no PE transposes are
needed.  Anchors are block-sharded 64 per core; the 8 partial
(sum, count) pairs are reduced on the host (the all-reduce of the
sharding hint).  Host-side prep is integer/mask/layout logic only; all
float arithmetic runs on device.
"""

import numpy as np

import concourse.bass as bass
import concourse.tile as tile
from concourse import bacc, mybir
from concourse.bass_utils import run_bass_kernel_spmd

F32 = mybir.dt.float32
F32R = mybir.dt.float32r
AF = mybir.ActivationFunctionType
OP = mybir.AluOpType

N = 512          # rows
D = 256          # embedding dim
NCORES = 8
A = N // NCORES  # anchors per core
MARGIN = 5.0
EPS = 1e-16
BIG = 1e30       # pad-pair kill value
BIGM = 128.0     # same-label mask / bias carrier (power of two)

_cache = {}


def _build(Tp: int):
    """Build the (uniform, SPMD) per-core Bass program for Tp pair tiles."""
    nc = bacc.Bacc(None, target_bir_lowering=False)

    rep_d = nc.declare_dram_parameter("rep", [N, D], F32, isOutput=False)
    rept_d = nc.declare_dram_parameter("rept", [128, 2, N], F32, isOutput=False)
    repa_d = nc.declare_dram_parameter("repa", [A, D], F32, isOutput=False)
    repat_d = nc.declare_dram_parameter("repat", [128, 2, A], F32, isOutput=False)
    bigm_d = nc.declare_dram_parameter("bigm", [A, N], F32, isOutput=False)
    sel_d = nc.declare_dram_parameter("sel", [A, Tp * 128], F32, isOutput=False)
    pm_d = nc.declare_dram_parameter("pm", [128, 2 * Tp], F32, isOutput=False)
    out_d = nc.declare_dram_parameter("out", [1, 2 * Tp], F32, isOutput=True)

    with tile.TileContext(nc) as tc:
        with (
            tc.tile_pool(name="singles", bufs=1) as sg,
            tc.tile_pool(name="scr", bufs=2) as scr,
            tc.tile_pool(name="xs", bufs=3) as xs,
            tc.tile_pool(name="dr", bufs=1, space="DRAM") as dr,
            tc.tile_pool(name="ppf", bufs=1, space="PSUM") as ppf,
            tc.tile_pool(name="ppg", bufs=4, space="PSUM") as ppg,
            tc.tile_pool(name="ppd", bufs=1, space="PSUM") as ppd,
        ):
            iota_f = sg.tile([128, N], F32)
            nc.gpsimd.iota(
                iota_f[:], [[1, N]], channel_multiplier=0,
                allow_small_or_imprecise_dtypes=True,
            )
            ones = sg.tile([128, 1], F32)
            nc.vector.memset(ones[:], 1.0)
            ones1 = sg.tile([1, A], F32)
            nc.vector.memset(ones1[:], 1.0)
            ones1r = sg.tile([1, A], F32R)
            nc.vector.tensor_copy(ones1r[:], ones1[:])

            # input loads, spread across HWDGE/SWDGE queues via issuing engine
            rept_s = sg.tile([128, 2, N], F32)     # rept[p, c, j] = rep[j, c*128+p]
            for q in range(4):
                eng = nc.sync if q % 2 == 0 else nc.scalar
                eng.dma_start(
                    rept_s[:, q // 2, (q % 2) * 256:(q % 2) * 256 + 256],
                    rept_d[:, q // 2, (q % 2) * 256:(q % 2) * 256 + 256],
                )
            rep_s = sg.tile([128, 4, D], F32)      # rep[t*128+p, :] -> rep_s[p, t, :]
            for t in range(4):
                eng = nc.sync if t % 2 == 0 else nc.scalar
                eng.dma_start(rep_s[:, t, :], rep_d[t * 128:(t + 1) * 128, :])
            repat_s = sg.tile([128, 2, A], F32)    # repat[p, c, a] = repa[a, c*128+p]
            nc.gpsimd.dma_start(repat_s[:], repat_d[:])
            repa_s = sg.tile([A, D], F32)
            nc.gpsimd.dma_start(repa_s[:], repa_d[:])
            bigm_s = sg.tile([A, N], F32)
            nc.gpsimd.dma_start(bigm_s[:], bigm_d[:])
            sel_s = sg.tile([A, Tp * 128], F32)
            nc.sync.dma_start(sel_s[:], sel_d[:])
            pm_s = sg.tile([128, 2 * Tp], F32)     # [:, :Tp] pidx, [:, Tp:] margin
            nc.scalar.dma_start(pm_s[:], pm_d[:])

            # float32r copies (PE consumes pre-rounded operands)
            reptr = sg.tile([128, 2, N], F32R)
            nc.vector.tensor_copy(reptr[:], rept_s[:])
            negTa = sg.tile([128, 2, A], F32R)
            nc.vector.tensor_scalar_mul(negTa[:], repat_s[:], -2.0)
            selr = sg.tile([A, Tp * 128], F32R)
            nc.vector.tensor_copy(selr[:], sel_s[:])

            # sq4[p, t] = ||rep_{t*128+p}||^2 -> sqrow[1, j] via DRAM bounce
            sq4 = sg.tile([128, 4], F32)
            for t in range(4):
                s4s = scr.tile([128, D], F32, tag="s4s")
                nc.vector.scalar_tensor_tensor(
                    out=s4s[:], in0=rep_s[:, t, :], scalar=1.0, in1=rep_s[:, t, :],
                    op0=OP.mult, op1=OP.mult, accum_out=sq4[:, t:t + 1],
                )
            sqd = dr.tile([4, 128], F32)
            nc.sync.dma_start(sqd.rearrange("t p -> p t"), sq4[:])
            sqrow = sg.tile([1, N], F32)
            nc.sync.dma_start(sqrow[:], sqd.rearrange("t p -> () (t p)"))
            sqrowr = sg.tile([1, N], F32R)
            nc.vector.tensor_copy(sqrowr[:], sqrow[:])

            # sq_anch[64,1] = ||rep_a||^2
            sqa_scr = scr.tile([A, D], F32, tag="sqa")
            sqanch = sg.tile([A, 1], F32)
            nc.vector.scalar_tensor_tensor(
                out=sqa_scr[:], in0=repa_s[:], scalar=1.0, in1=repa_s[:],
                op0=OP.mult, op1=OP.mult, accum_out=sqanch[:],
            )

            # d2[a, j] = sq_a + sq_j - 2*dot  (aug matmul + bias add)
            d2_p = ppd.tile([A, N], F32, tag="d2")
            nc.tensor.matmul(d2_p[:], negTa[:, 0, :], reptr[:, 0, :],
                             start=True, stop=False)
            nc.tensor.matmul(d2_p[:], negTa[:, 1, :], reptr[:, 1, :],
                             start=False, stop=False)
            nc.tensor.matmul(d2_p[:], ones1r[:], sqrowr[:], start=False, stop=True)

            d2c = sg.tile([A, N], F32)
            nc.vector.tensor_scalar(
                d2c[:], d2_p[:], sqanch[:], 0.0, OP.add, OP.max
            )

            # ym = sqrt(d2c) + BIGM*same  (written rounded for the gather)
            dtmp = scr.tile([A, N], F32, tag="dtmp")
            nc.scalar.activation(dtmp[:], d2c[:], AF.Sqrt)
            ym = sg.tile([A, N], F32R)
            nc.vector.tensor_add(ym[:], bigm_s[:], dtmp[:])

            # pair tiles
            SC = sg.tile([128, 2 * Tp], F32)
            for t in range(Tp):
                gy = ppg.tile([128, N], F32, tag="gy")
                nc.tensor.matmul(gy[:], selr[:, t * 128:(t + 1) * 128], ym[:],
                                 start=True, stop=True)

                stt = scr.tile([128, N], F32, tag="stt")
                xv = xs.tile([128, 1], F32, tag="xv")
                nc.vector.scalar_tensor_tensor(
                    out=stt[:], in0=iota_f[:], scalar=pm_s[:, t:t + 1], in1=gy[:],
                    op0=OP.is_equal, op1=OP.mult, accum_out=xv[:],
                )
                xp = xs.tile([128, 1], F32, tag="xp")
                nc.vector.tensor_scalar(
                    xp[:], xv[:], pm_s[:, Tp + t:Tp + t + 1], None, OP.add
                )

                rel = scr.tile([128, N], F32, tag="rel")
                nc.scalar.activation(
                    rel[:], gy[:], AF.Relu, bias=xp[:], scale=-1.0,
                    accum_out=SC[:, t:t + 1],
                )
                cnt = scr.tile([128, N], F32, tag="cnt")
                nc.vector.tensor_scalar(
                    cnt[:], gy[:], xp[:], 0.0, OP.is_lt, OP.add,
                    accum_out=SC[:, Tp + t:Tp + t + 1],
                )

            # partition-sum S and C columns -> [1, 2*Tp]
            fin_p = ppf.tile([1, 2 * Tp], F32, tag="fin")
            nc.tensor.matmul(fin_p[:], ones[:], SC[:], start=True, stop=True)
            outsb = sg.tile([1, 2 * Tp], F32)
            nc.vector.tensor_copy(outsb[:], fin_p[:])
            nc.sync.dma_start(out_d[:], outsb[:])

    nc.finalize()
    return nc


def _prep(rep: np.ndarray, labels: np.ndarray):
    """Host-side integer/mask/layout prep: shard anchors, enumerate pairs."""
    rep = np.ascontiguousarray(np.asarray(rep, dtype=np.float32))
    labels = np.asarray(labels)
    same = labels[:, None] == labels[None, :]

    # rep.T packed [128, 2, N]: rept[p, c, j] = rep[j, c*128 + p]
    rept = np.ascontiguousarray(
        rep.T.reshape(2, 128, N).transpose(1, 0, 2)
    )

    pairs = []
    for c in range(NCORES):
        base = c * A
        prs = [
            (j, p)
            for j in range(A)
            for p in np.nonzero(same[base + j])[0]
            if p != base + j
        ]
        pairs.append(prs)
    Tp = max(1, max((len(p) + 127) // 128 for p in pairs))

    in_maps = []
    for c in range(NCORES):
        base = c * A
        repa = rep[base:base + A]
        repat = np.ascontiguousarray(
            repa.T.reshape(2, 128, A).transpose(1, 0, 2)
        )
        bigm = np.where(same[base:base + A], BIGM, 0.0).astype(np.float32)
        sel = np.zeros((A, Tp * 128), np.float32)
        pm = np.zeros((128, 2 * Tp), np.float32)
        pm[:, Tp:] = -BIG
        for i, (j, p) in enumerate(pairs[c]):
            t, r = divmod(i, 128)
            sel[j, i] = 1.0
            pm[r, t] = p
            pm[r, Tp + t] = MARGIN - BIGM
        in_maps.append({
            "rep": rep,
            "rept": rept,
            "repa": repa,
            "repat": repat,
            "bigm": bigm,
            "sel": sel,
            "pm": pm,
        })
    return Tp, in_maps


def _run(rep, labels, trace=False):
    Tp, in_maps = _prep(rep, labels)
    if Tp not in _cache:
        _cache[Tp] = _build(Tp)
    nc = _cache[Tp]
    res = run_bass_kernel_spmd(nc, in_maps, list(range(NCORES)), trace=trace)
    outs = np.stack([res.results[c]["out"][0] for c in range(NCORES)])  # [8, 2*Tp]
    S = float(outs[:, :Tp].sum())
    C = float(outs[:, Tp:].sum())
    loss = np.float32(S / (C + EPS))
    return np.asarray(loss, dtype=np.float32), res


def kernel(rep, labels):
    loss, _ = _run(rep, labels, trace=False)
    return loss


# revision 17
# speedup vs baseline: 1.3796x; 1.3796x over previous
"""BatchAllTripletLoss kernel for 8 Trainium2 NeuronCores.

Reference computation:
    pd = pairwise_euclidean(rep)                        # [512, 512]
    tl[a,p,k] = relu(pd[a,p] - pd[a,k] + 5.0) * mask    # [512, 512, 512]
    loss = sum(tl) / (count(tl > eps) + eps)

The mask (p!=a, k!=a, p!=k, label[p]==label[a], label[k]!=label[a])
collapses: label[p]==label[a] and label[k]!=label[a] imply p!=k and k!=a,
so valid triplets are exactly (anchor-positive pairs) x (k with a
different label).  With 64 labels over 512 rows there are only ~4100
(a,p) pairs, so instead of a dense [N,N,N] sweep each core processes its
anchors' pairs as rows of [128-pair, 512-k] tiles:

  per core (64 anchors):
    d[64,512]   = sqrt(relu(aug-matmul))            PE + DVE + ACT
    ym          = d + BIGM*same_label               DVE
    per pair-tile t:
      Gym       = sel_t.T @ ym                      PE one-hot row gather
      x[p]      = sum_k (iota==pidx)*Gym            DVE; = d[a,p] + BIGM
      xp        = x + (margin - BIGM)               DVE
      S_t[p]    = sum_k relu(xp - Gym)              ACT accum
      C_t[p]    = sum_k (Gym < xp)                  DVE accum
    out[1,2*Tp] = ones.T @ [S | C]                  PE partition sum

All matmuls run in float32r (single-pass fp32, ~2^-13 relative rounding;
the one-hot gather then carries that rounding into d).  BIGM = 128 both
masks out same-label k columns (xp <= ~35 << 128 so relu/count give
exactly 0) and carries the bias through the gather; the combined
rounding is ~1e-2 absolute per term, mean-zero, ~1e-4 on the final sums.
rep arrives both row-major (for the row-norm accumulates) and
host-transposed (pure layout permutation) so no PE transposes are
needed.  Anchors are block-sharded 64 per core; the 8 partial
(sum, count) pairs are reduced on the host (the all-reduce of the
sharding hint).  Host-side prep is integer/mask/layout logic only; all
float arithmetic runs on device.
"""

import numpy as np

import concourse.bass as bass
import concourse.tile as tile
from concourse import bacc, mybir
from concourse.bass_utils import run_bass_kernel_spmd
from concourse.masks import make_identity

F32 = mybir.dt.float32
F32R = mybir.dt.float32r
AF = mybir.ActivationFunctionType
OP = mybir.AluOpType

N = 512          # rows
D = 256          # embedding dim
NCORES = 8
A = N // NCORES  # anchors per core
MARGIN = 5.0
EPS = 1e-16
BIG = 1e30       # pad-pair kill value
BIGM = 128.0     # same-label mask / bias carrier (power of two)

_cache = {}


def _build(Tp: int):
    """Build the (uniform, SPMD) per-core Bass program for Tp pair tiles."""
    nc = bacc.Bacc(None, target_bir_lowering=False)

    rep_d = nc.declare_dram_parameter("rep", [N, D], F32, isOutput=False)
    rept_d = nc.declare_dram_parameter("rept", [128, 2, N], F32, isOutput=False)
    repa_d = nc.declare_dram_parameter("repa", [A, D], F32, isOutput=False)
    repat_d = nc.declare_dram_parameter("repat", [128, 2, A], F32, isOutput=False)
    bigm_d = nc.declare_dram_parameter("bigm", [A, N], F32, isOutput=False)
    sel_d = nc.declare_dram_parameter("sel", [A, Tp * 128], F32, isOutput=False)
    pm_d = nc.declare_dram_parameter("pm", [128, 2 * Tp], F32, isOutput=False)
    out_d = nc.declare_dram_parameter("out", [1, 2 * Tp], F32, isOutput=True)

    with tile.TileContext(nc) as tc:
        with (
            tc.tile_pool(name="singles", bufs=1) as sg,
            tc.tile_pool(name="scr", bufs=2) as scr,
            tc.tile_pool(name="xs", bufs=3) as xs,
            tc.tile_pool(name="ppf", bufs=1, space="PSUM") as ppf,
            tc.tile_pool(name="ppg", bufs=4, space="PSUM") as ppg,
            tc.tile_pool(name="ppd", bufs=1, space="PSUM") as ppd,
        ):
            ident = sg.tile([128, 128], F32)
            make_identity(nc, ident[:])
            iota_f = sg.tile([128, N], F32)
            nc.gpsimd.iota(
                iota_f[:], [[1, N]], channel_multiplier=0,
                allow_small_or_imprecise_dtypes=True,
            )
            ones = sg.tile([128, 1], F32)
            nc.vector.memset(ones[:], 1.0)
            ones1 = sg.tile([1, A], F32)
            nc.vector.memset(ones1[:], 1.0)
            ones1r = sg.tile([1, A], F32R)
            nc.vector.tensor_copy(ones1r[:], ones1[:])

            # input loads, spread across the two HWDGE queues; rep first
            # (the row-norm chain below is the longest dependency chain)
            rep_s = sg.tile([128, 4, D], F32)      # rep[t*128+p, :] -> rep_s[p, t, :]
            for t in range(4):
                eng = nc.sync if t % 2 == 0 else nc.scalar
                eng.dma_start(rep_s[:, t, :], rep_d[t * 128:(t + 1) * 128, :])
            rept_s = sg.tile([128, 2, N], F32)     # rept[p, c, j] = rep[j, c*128+p]
            for q in range(4):
                eng = nc.sync if q % 2 == 0 else nc.scalar
                eng.dma_start(
                    rept_s[:, q // 2, (q % 2) * 256:(q % 2) * 256 + 256],
                    rept_d[:, q // 2, (q % 2) * 256:(q % 2) * 256 + 256],
                )
            repat_s = sg.tile([128, 2, A], F32)    # repat[p, c, a] = repa[a, c*128+p]
            nc.sync.dma_start(repat_s[:], repat_d[:])
            repa_s = sg.tile([A, D], F32)
            nc.scalar.dma_start(repa_s[:], repa_d[:])
            bigm_s = sg.tile([A, N], F32)
            nc.sync.dma_start(bigm_s[:], bigm_d[:])
            sel_s = sg.tile([A, Tp * 128], F32)
            nc.scalar.dma_start(sel_s[:], sel_d[:])
            pm_s = sg.tile([128, 2 * Tp], F32)     # [:, :Tp] pidx, [:, Tp:] margin
            nc.sync.dma_start(pm_s[:], pm_d[:])

            # sq4[p, t] = ||rep_{t*128+p}||^2 -> sqrow[1, j] via PE column
            # transposes (emitted first: this chain gates the d2 matmul)
            sq4 = sg.tile([128, 4], F32)
            for t in range(4):
                s4s = scr.tile([128, D], F32, tag="s4s")
                nc.vector.scalar_tensor_tensor(
                    out=s4s[:], in0=rep_s[:, t, :], scalar=1.0, in1=rep_s[:, t, :],
                    op0=OP.mult, op1=OP.mult, accum_out=sq4[:, t:t + 1],
                )
            sqrow_p = ppf.tile([1, N], F32, tag="fin")
            for t in range(4):
                nc.tensor.transpose(
                    sqrow_p[0:1, t * 128:(t + 1) * 128], sq4[:, t:t + 1], ident[:]
                )
            sqrowr = sg.tile([1, N], F32R)
            nc.vector.tensor_copy(sqrowr[:], sqrow_p[:])

            # sq_anch[64,1] = ||rep_a||^2
            sqa_scr = scr.tile([A, D], F32, tag="sqa")
            sqanch = sg.tile([A, 1], F32)
            nc.vector.scalar_tensor_tensor(
                out=sqa_scr[:], in0=repa_s[:], scalar=1.0, in1=repa_s[:],
                op0=OP.mult, op1=OP.mult, accum_out=sqanch[:],
            )

            # float32r copies (PE consumes pre-rounded operands)
            reptr = sg.tile([128, 2, N], F32R)
            nc.vector.tensor_copy(reptr[:], rept_s[:])
            negTa = sg.tile([128, 2, A], F32R)
            nc.vector.tensor_scalar_mul(negTa[:], repat_s[:], -2.0)
            selr = sg.tile([A, Tp * 128], F32R)
            nc.vector.tensor_copy(selr[:], sel_s[:])

            # d2[a, j] = sq_a + sq_j - 2*dot  (aug matmul + bias add)
            d2_p = ppd.tile([A, N], F32, tag="d2")
            nc.tensor.matmul(d2_p[:], negTa[:, 0, :], reptr[:, 0, :],
                             start=True, stop=False)
            nc.tensor.matmul(d2_p[:], negTa[:, 1, :], reptr[:, 1, :],
                             start=False, stop=False)
            nc.tensor.matmul(d2_p[:], ones1r[:], sqrowr[:], start=False, stop=True)

            d2c = sg.tile([A, N], F32)
            nc.vector.tensor_scalar(
                d2c[:], d2_p[:], sqanch[:], 0.0, OP.add, OP.max
            )

            # ym = sqrt(d2c) + BIGM*same  (written rounded for the gather)
            dtmp = scr.tile([A, N], F32, tag="dtmp")
            nc.scalar.activation(dtmp[:], d2c[:], AF.Sqrt)
            ym = sg.tile([A, N], F32R)
            nc.vector.tensor_add(ym[:], bigm_s[:], dtmp[:])

            # pair tiles
            SC = sg.tile([128, 2 * Tp], F32)
            for t in range(Tp):
                gy = ppg.tile([128, N], F32, tag="gy")
                nc.tensor.matmul(gy[:], selr[:, t * 128:(t + 1) * 128], ym[:],
                                 start=True, stop=True)

                stt = scr.tile([128, N], F32, tag="stt")
                xv = xs.tile([128, 1], F32, tag="xv")
                nc.vector.scalar_tensor_tensor(
                    out=stt[:], in0=iota_f[:], scalar=pm_s[:, t:t + 1], in1=gy[:],
                    op0=OP.is_equal, op1=OP.mult, accum_out=xv[:],
                )
                xp = xs.tile([128, 1], F32, tag="xp")
                nc.vector.tensor_scalar(
                    xp[:], xv[:], pm_s[:, Tp + t:Tp + t + 1], None, OP.add
                )

                rel = scr.tile([128, N], F32, tag="rel")
                nc.scalar.activation(
                    rel[:], gy[:], AF.Relu, bias=xp[:], scale=-1.0,
                    accum_out=SC[:, t:t + 1],
                )
                cnt = scr.tile([128, N], F32, tag="cnt")
                nc.vector.tensor_scalar(
                    cnt[:], gy[:], xp[:], 0.0, OP.is_lt, OP.add,
                    accum_out=SC[:, Tp + t:Tp + t + 1],
                )

            # partition-sum S and C columns -> [1, 2*Tp]
            fin_p = ppf.tile([1, 2 * Tp], F32, tag="fin")
            nc.tensor.matmul(fin_p[:], ones[:], SC[:], start=True, stop=True)
            outsb = sg.tile([1, 2 * Tp], F32)
            nc.vector.tensor_copy(outsb[:], fin_p[:])
            nc.sync.dma_start(out_d[:], outsb[:])

    nc.finalize()
    return nc


def _prep(rep: np.ndarray, labels: np.ndarray):
    """Host-side integer/mask/layout prep: shard anchors, enumerate pairs."""
    rep = np.ascontiguousarray(np.asarray(rep, dtype=np.float32))
    labels = np.asarray(labels)
    same = labels[:, None] == labels[None, :]

    # rep.T packed [128, 2, N]: rept[p, c, j] = rep[j, c*128 + p]
    rept = np.ascontiguousarray(
        rep.T.reshape(2, 128, N).transpose(1, 0, 2)
    )

    pairs = []
    for c in range(NCORES):
        base = c * A
        prs = [
            (j, p)
            for j in range(A)
            for p in np.nonzero(same[base + j])[0]
            if p != base + j
        ]
        pairs.append(prs)
    Tp = max(1, max((len(p) + 127) // 128 for p in pairs))

    in_maps = []
    for c in range(NCORES):
        base = c * A
        repa = rep[base:base + A]
        repat = np.ascontiguousarray(
            repa.T.reshape(2, 128, A).transpose(1, 0, 2)
        )
        bigm = np.where(same[base:base + A], BIGM, 0.0).astype(np.float32)
        sel = np.zeros((A, Tp * 128), np.float32)
        pm = np.zeros((128, 2 * Tp), np.float32)
        pm[:, Tp:] = -BIG
        for i, (j, p) in enumerate(pairs[c]):
            t, r = divmod(i, 128)
            sel[j, i] = 1.0
            pm[r, t] = p
            pm[r, Tp + t] = MARGIN - BIGM
        in_maps.append({
            "rep": rep,
            "rept": rept,
            "repa": repa,
            "repat": repat,
            "bigm": bigm,
            "sel": sel,
            "pm": pm,
        })
    return Tp, in_maps


def _run(rep, labels, trace=False):
    Tp, in_maps = _prep(rep, labels)
    if Tp not in _cache:
        _cache[Tp] = _build(Tp)
    nc = _cache[Tp]
    res = run_bass_kernel_spmd(nc, in_maps, list(range(NCORES)), trace=trace)
    outs = np.stack([res.results[c]["out"][0] for c in range(NCORES)])  # [8, 2*Tp]
    S = float(outs[:, :Tp].sum())
    C = float(outs[:, Tp:].sum())
    loss = np.float32(S / (C + EPS))
    return np.asarray(loss, dtype=np.float32), res


def kernel(rep, labels):
    loss, _ = _run(rep, labels, trace=False)
    return loss


# revision 18
# speedup vs baseline: 1.4883x; 1.0787x over previous
"""BatchAllTripletLoss kernel for 8 Trainium2 NeuronCores.

Reference computation:
    pd = pairwise_euclidean(rep)                        # [512, 512]
    tl[a,p,k] = relu(pd[a,p] - pd[a,k] + 5.0) * mask    # [512, 512, 512]
    loss = sum(tl) / (count(tl > eps) + eps)

The mask (p!=a, k!=a, p!=k, label[p]==label[a], label[k]!=label[a])
collapses: label[p]==label[a] and label[k]!=label[a] imply p!=k and k!=a,
so valid triplets are exactly (anchor-positive pairs) x (k with a
different label).  With 64 labels over 512 rows there are only ~4100
(a,p) pairs, so instead of a dense [N,N,N] sweep each core processes its
anchors' pairs as rows of [128-pair, 512-k] tiles:

  per core (64 anchors):
    d[64,512]   = sqrt(relu(aug-matmul))            PE + DVE + ACT
    ym          = d + BIGM*same_label               DVE
    per pair-tile t:
      Gym       = sel_t.T @ ym                      PE one-hot row gather
      x[p]      = sum_k (iota==pidx)*Gym            DVE; = d[a,p] + BIGM
      xp        = x + (margin - BIGM)               DVE
      S_t[p]    = sum_k relu(xp - Gym)              ACT accum
      C_t[p]    = sum_k (Gym < xp)                  DVE accum
    out[1,2*Tp] = ones.T @ [S | C]                  PE partition sum

All matmuls run in float32r (single-pass fp32, ~2^-13 relative rounding;
the one-hot gather then carries that rounding into d).  BIGM = 128 both
masks out same-label k columns (xp <= ~35 << 128 so relu/count give
exactly 0) and carries the bias through the gather; the combined
rounding is ~1e-2 absolute per term, mean-zero, ~1e-4 on the final sums.
rep arrives both row-major (for the row-norm accumulates) and
host-transposed (pure layout permutation) so no PE transposes are
needed.  Anchors are block-sharded 64 per core; the 8 partial
(sum, count) pairs are reduced on the host (the all-reduce of the
sharding hint).  Host-side prep is integer/mask/layout logic only; all
float arithmetic runs on device.
"""

import numpy as np

import concourse.bass as bass
import concourse.tile as tile
from concourse import bacc, mybir
from concourse.bass_utils import run_bass_kernel_spmd

F32 = mybir.dt.float32
F32R = mybir.dt.float32r
AF = mybir.ActivationFunctionType
OP = mybir.AluOpType

N = 512          # rows
D = 256          # embedding dim
NCORES = 8
A = N // NCORES  # anchors per core
MARGIN = 5.0
EPS = 1e-16
BIG = 1e30       # pad-pair kill value
BIGM = 128.0     # same-label mask / bias carrier (power of two)

_cache = {}


def _build(Tp: int):
    """Build the (uniform, SPMD) per-core Bass program for Tp pair tiles."""
    nc = bacc.Bacc(None, target_bir_lowering=False)

    rept_d = nc.declare_dram_parameter("rept", [128, 2, N], F32, isOutput=False)
    repa_d = nc.declare_dram_parameter("repa", [A, D], F32, isOutput=False)
    repat_d = nc.declare_dram_parameter("repat", [128, 2, A], F32, isOutput=False)
    bigm_d = nc.declare_dram_parameter("bigm", [A, N], F32, isOutput=False)
    sel_d = nc.declare_dram_parameter("sel", [A, Tp * 128], F32, isOutput=False)
    pm_d = nc.declare_dram_parameter("pm", [128, 2 * Tp], F32, isOutput=False)
    out_d = nc.declare_dram_parameter("out", [1, 2 * Tp], F32, isOutput=True)

    with tile.TileContext(nc) as tc:
        with (
            tc.tile_pool(name="singles", bufs=1) as sg,
            tc.tile_pool(name="scr", bufs=2) as scr,
            tc.tile_pool(name="xs", bufs=3) as xs,
            tc.tile_pool(name="ppf", bufs=1, space="PSUM") as ppf,
            tc.tile_pool(name="ppg", bufs=4, space="PSUM") as ppg,
            tc.tile_pool(name="ppd", bufs=1, space="PSUM") as ppd,
        ):
            iota_f = sg.tile([128, N], F32)
            nc.gpsimd.iota(
                iota_f[:], [[1, N]], channel_multiplier=0,
                allow_small_or_imprecise_dtypes=True,
            )
            ones = sg.tile([128, 1], F32)
            nc.vector.memset(ones[:], 1.0)
            onesr = sg.tile([128, 1], F32R)
            nc.vector.tensor_copy(onesr[:], ones[:])
            ones1 = sg.tile([1, A], F32)
            nc.vector.memset(ones1[:], 1.0)
            ones1r = sg.tile([1, A], F32R)
            nc.vector.tensor_copy(ones1r[:], ones1[:])

            # input loads, spread across the two HWDGE queues; rep first
            # (the row-norm chain below is the longest dependency chain)
            rept_s = sg.tile([128, 2, N], F32)     # rept[p, c, j] = rep[j, c*128+p]
            for q in range(4):
                eng = nc.sync if q % 2 == 0 else nc.scalar
                eng.dma_start(
                    rept_s[:, q // 2, (q % 2) * 256:(q % 2) * 256 + 256],
                    rept_d[:, q // 2, (q % 2) * 256:(q % 2) * 256 + 256],
                )
            repat_s = sg.tile([128, 2, A], F32)    # repat[p, c, a] = repa[a, c*128+p]
            nc.sync.dma_start(repat_s[:], repat_d[:])
            repa_s = sg.tile([A, D], F32)
            nc.scalar.dma_start(repa_s[:], repa_d[:])
            bigm_s = sg.tile([A, N], F32)
            nc.sync.dma_start(bigm_s[:], bigm_d[:])
            sel_s = sg.tile([A, Tp * 128], F32)
            nc.scalar.dma_start(sel_s[:], sel_d[:])
            pm_s = sg.tile([128, 2 * Tp], F32)     # [:, :Tp] pidx, [:, Tp:] margin
            nc.sync.dma_start(pm_s[:], pm_d[:])

            # float32r operand copies (PE consumes pre-rounded data), per
            # chunk so each overlaps the other chunk's DMA
            reptr = sg.tile([128, 2, N], F32R)
            for c in range(2):
                nc.vector.tensor_copy(reptr[:, c, :], rept_s[:, c, :])

            # sq_row[1, j] = ||rep_j||^2 = ones.T @ (rept * rept)
            sqsq = sg.tile([128, 2, N], F32R)
            for c in range(2):
                nc.vector.tensor_mul(sqsq[:, c, :], rept_s[:, c, :], rept_s[:, c, :])
            sqrow_p = ppf.tile([1, N], F32, tag="fin")
            nc.tensor.matmul(sqrow_p[:], onesr[:], sqsq[:, 0, :], start=True, stop=False)
            nc.tensor.matmul(sqrow_p[:], onesr[:], sqsq[:, 1, :], start=False, stop=True)
            sqrowr = sg.tile([1, N], F32R)
            nc.vector.tensor_copy(sqrowr[:], sqrow_p[:])

            # sq_anch[64,1] = ||rep_a||^2
            sqa_scr = scr.tile([A, D], F32, tag="sqa")
            sqanch = sg.tile([A, 1], F32)
            nc.vector.scalar_tensor_tensor(
                out=sqa_scr[:], in0=repa_s[:], scalar=1.0, in1=repa_s[:],
                op0=OP.mult, op1=OP.mult, accum_out=sqanch[:],
            )

            negTa = sg.tile([128, 2, A], F32R)
            nc.vector.tensor_scalar_mul(negTa[:], repat_s[:], -2.0)
            selr = sg.tile([A, Tp * 128], F32R)
            nc.vector.tensor_copy(selr[:], sel_s[:])

            # d2[a, j] = sq_a + sq_j - 2*dot  (aug matmul + bias add)
            d2_p = ppd.tile([A, N], F32, tag="d2")
            nc.tensor.matmul(d2_p[:], negTa[:, 0, :], reptr[:, 0, :],
                             start=True, stop=False)
            nc.tensor.matmul(d2_p[:], negTa[:, 1, :], reptr[:, 1, :],
                             start=False, stop=False)
            nc.tensor.matmul(d2_p[:], ones1r[:], sqrowr[:], start=False, stop=True)

            d2c = sg.tile([A, N], F32)
            nc.vector.tensor_scalar(
                d2c[:], d2_p[:], sqanch[:], 0.0, OP.add, OP.max
            )

            # ym = sqrt(d2c) + BIGM*same  (written rounded for the gather)
            dtmp = scr.tile([A, N], F32, tag="dtmp")
            nc.scalar.activation(dtmp[:], d2c[:], AF.Sqrt)
            ym = sg.tile([A, N], F32R)
            nc.vector.tensor_add(ym[:], bigm_s[:], dtmp[:])

            # pair tiles
            SC = sg.tile([128, 2 * Tp], F32)
            for t in range(Tp):
                gy = ppg.tile([128, N], F32, tag="gy")
                nc.tensor.matmul(gy[:], selr[:, t * 128:(t + 1) * 128], ym[:],
                                 start=True, stop=True)

                stt = scr.tile([128, N], F32, tag="stt")
                xv = xs.tile([128, 1], F32, tag="xv")
                nc.vector.scalar_tensor_tensor(
                    out=stt[:], in0=iota_f[:], scalar=pm_s[:, t:t + 1], in1=gy[:],
                    op0=OP.is_equal, op1=OP.mult, accum_out=xv[:],
                )
                xp = xs.tile([128, 1], F32, tag="xp")
                nc.vector.tensor_scalar(
                    xp[:], xv[:], pm_s[:, Tp + t:Tp + t + 1], None, OP.add
                )

                rel = scr.tile([128, N], F32, tag="rel")
                nc.scalar.activation(
                    rel[:], gy[:], AF.Relu, bias=xp[:], scale=-1.0,
                    accum_out=SC[:, t:t + 1],
                )
                cnt = scr.tile([128, N], F32, tag="cnt")
                nc.vector.tensor_scalar(
                    cnt[:], rel[:], 0.0, 0.0, OP.is_gt, OP.add,
                    accum_out=SC[:, Tp + t:Tp + t + 1],
                )

            # partition-sum S and C columns -> [1, 2*Tp]
            fin_p = ppf.tile([1, 2 * Tp], F32, tag="fin")
            nc.tensor.matmul(fin_p[:], ones[:], SC[:], start=True, stop=True)
            outsb = sg.tile([1, 2 * Tp], F32)
            nc.vector.tensor_copy(outsb[:], fin_p[:])
            nc.sync.dma_start(out_d[:], outsb[:])

    nc.finalize()
    return nc


def _prep(rep: np.ndarray, labels: np.ndarray):
    """Host-side integer/mask/layout prep: shard anchors, enumerate pairs."""
    rep = np.ascontiguousarray(np.asarray(rep, dtype=np.float32))
    labels = np.asarray(labels)
    same = labels[:, None] == labels[None, :]

    # rep.T packed [128, 2, N]: rept[p, c, j] = rep[j, c*128 + p]
    rept = np.ascontiguousarray(
        rep.T.reshape(2, 128, N).transpose(1, 0, 2)
    )

    pairs = []
    for c in range(NCORES):
        base = c * A
        prs = [
            (j, p)
            for j in range(A)
            for p in np.nonzero(same[base + j])[0]
            if p != base + j
        ]
        pairs.append(prs)
    Tp = max(1, max((len(p) + 127) // 128 for p in pairs))

    in_maps = []
    for c in range(NCORES):
        base = c * A
        repa = rep[base:base + A]
        repat = np.ascontiguousarray(
            repa.T.reshape(2, 128, A).transpose(1, 0, 2)
        )
        bigm = np.where(same[base:base + A], BIGM, 0.0).astype(np.float32)
        sel = np.zeros((A, Tp * 128), np.float32)
        pm = np.zeros((128, 2 * Tp), np.float32)
        pm[:, Tp:] = -BIG
        for i, (j, p) in enumerate(pairs[c]):
            t, r = divmod(i, 128)
            sel[j, i] = 1.0
            pm[r, t] = p
            pm[r, Tp + t] = MARGIN - BIGM
        in_maps.append({
            "rept": rept,
            "repa": repa,
            "repat": repat,
            "bigm": bigm,
            "sel": sel,
            "pm": pm,
        })
    return Tp, in_maps


def _run(rep, labels, trace=False):
    Tp, in_maps = _prep(rep, labels)
    if Tp not in _cache:
        _cache[Tp] = _build(Tp)
    nc = _cache[Tp]
    res = run_bass_kernel_spmd(nc, in_maps, list(range(NCORES)), trace=trace)
    outs = np.stack([res.results[c]["out"][0] for c in range(NCORES)])  # [8, 2*Tp]
    S = float(outs[:, :Tp].sum())
    C = float(outs[:, Tp:].sum())
    loss = np.float32(S / (C + EPS))
    return np.asarray(loss, dtype=np.float32), res


def kernel(rep, labels):
    loss, _ = _run(rep, labels, trace=False)
    return loss
